# revision 5
# baseline (speedup 1.0000x reference)
"""BiDAF attention-flow layer on 8 Trainium2 NeuronCores.

Data-parallel over batch: each core processes B/8 = 8 batches.

Math (per batch b):
  s[t,j] = h[t]·w_h + u[j]·w_u + (h[t]*w_hu)·u[j] + const
  a      = softmax_j(s)            -> only needs  sj = shu + su  (row consts cancel)
  c2q    = a @ u
  bt     = softmax_t(max_j s)      -> needs  m + sh  where m = max_j(sj)
  q2c    = bt @ h
  g      = [h | c2q | h*c2q | h*q2c]

The rank-1 bias terms b_h/b_u/b_hu shift every s[t,j] equally and cancel in
both softmaxes, so they are accepted but unused.

Layout per batch (core-local):
  htile [128, 7*200]   h rows chunked by 128 (chunk c at cols c*200..)
  hT    2x [100, 800]  h transposed (PE transpose), K-chunks of D
  s_ps  [128, 51]      cols 0:50 = shu+su (via K=1 ones-matmul), col 50 = sh
  softmax on free dim; p transposed back (PE) for the c2q matmul;
  y[1,200] = sum_t e_t h_t accumulated in PSUM over chunks; q2c = y/sum(e).
"""
import sys

if '/opt/trn_rl_repo' not in sys.path:
    sys.path.insert(0, '/opt/trn_rl_repo')

import numpy as np

B, T, J, D = 64, 800, 50, 200
NCORES = 8
BC = B // NCORES            # batches per core
P = 128
TCHUNKS = [(c * P, min(P, T - c * P)) for c in range((T + P - 1) // P)]
KCHUNKS = [(0, 100), (100, 100)]

_cache = {}


def _split_multi_waits(nc, max_waits=1):
    """This walrus build accepts at most one sync-wait per instruction.
    For any instruction carrying more, move the extra waits onto pure-wait
    EventSemaphore carriers inserted just before it on the same engine —
    the sequencer dispatches in order, so the blocking behavior is
    identical."""
    from concourse import mybir
    import bass_rust
    n = 0
    for f in nc.m.functions:
        for blk in f.blocks:
            insts = blk.instructions
            i = 0
            while i < len(insts):
                inst = insts[i]
                si = inst.sync_info
                if si is not None and len(si.on_wait) > max_waits:
                    waits = list(si.on_wait)
                    keep = waits[-max_waits:]
                    new = []
                    for w in waits[:-max_waits]:
                        d = mybir.InstEventSemaphore(
                            name=f"{inst.name}-sw{n}", ins=[], outs=[])
                        n += 1
                        d.engine = inst.engine
                        d.sync_info = bass_rust.SyncInfo(on_wait=[w], on_update=[])
                        new.append(d)
                    inst.sync_info = bass_rust.SyncInfo(
                        on_wait=keep, on_update=list(si.on_update))
                    for j, d in enumerate(new):
                        insts.insert(i + j, d)
                    i += len(new)
                i += 1
    return n


def _build():
    import concourse.bass as bass
    import concourse.tile as tile
    from concourse import mybir, masks
    from contextlib import ExitStack

    f32 = mybir.dt.float32
    AF = mybir.ActivationFunctionType
    AX = mybir.AxisListType

    nc = bass.Bass()
    h_in = nc.declare_dram_parameter("h", [BC, T, D], f32, isOutput=False)
    u_in = nc.declare_dram_parameter("u", [BC, J, D], f32, isOutput=False)
    wh_in = nc.declare_dram_parameter("w_h", [D], f32, isOutput=False)
    wu_in = nc.declare_dram_parameter("w_u", [D], f32, isOutput=False)
    whu_in = nc.declare_dram_parameter("w_hu", [D], f32, isOutput=False)
    g_out = nc.declare_dram_parameter("g", [BC, T, 4 * D], f32, isOutput=True)

    with tile.TileContext(nc) as tc, ExitStack() as ctx:
        singles = ctx.enter_context(tc.tile_pool(name="singles", bufs=1))
        hpool = ctx.enter_context(tc.tile_pool(name="hpool", bufs=2))
        hTpool = ctx.enter_context(tc.tile_pool(name="hTpool", bufs=2))
        upool = ctx.enter_context(tc.tile_pool(name="upool", bufs=2))
        bsmall = ctx.enter_context(tc.tile_pool(name="bsmall", bufs=2))
        csmall = ctx.enter_context(tc.tile_pool(name="csmall", bufs=3))
        gpool = ctx.enter_context(tc.tile_pool(name="gpool", bufs=4))
        hqpool = ctx.enter_context(tc.tile_pool(name="hqpool", bufs=4))
        ps_s = ctx.enter_context(
            tc.tile_pool(name="ps_s", bufs=2, space=bass.MemorySpace.PSUM))
        ps_tp = ctx.enter_context(
            tc.tile_pool(name="ps_tp", bufs=2, space=bass.MemorySpace.PSUM))
        ps_c2q = ctx.enter_context(
            tc.tile_pool(name="ps_c2q", bufs=2, space=bass.MemorySpace.PSUM))
        ps_acc = ctx.enter_context(
            tc.tile_pool(name="ps_acc", bufs=2, space=bass.MemorySpace.PSUM))

        # ---- once-per-core constants ----
        identity = singles.tile([P, P], f32)
        masks.make_identity(nc, identity[:])
        ones_row = singles.tile([1, P], f32)
        nc.vector.memset(ones_row, 1.0)
        ones_col = singles.tile([P, 1], f32)
        nc.vector.memset(ones_col, 1.0)

        wcols = {}
        for wname, src in (("wh", wh_in), ("wu", wu_in), ("whu", whu_in)):
            for kc, (d0, kn) in enumerate(KCHUNKS):
                t_ = singles.tile([kn, 1], f32, tag=f"{wname}{kc}")
                nc.sync.dma_start(out=t_[:, :], in_=src[d0:d0 + kn].rearrange("(p one) -> p one", one=1))
                wcols[(wname, kc)] = t_

        for b in range(BC):
            # ---- load h, u ----
            htile = hpool.tile([P, 7 * D], f32, tag="h")
            nmain = 6
            nc.sync.dma_start(
                out=htile[:, 0:nmain * D].rearrange("p (n d) -> p n d", d=D),
                in_=h_in[b, 0:nmain * P, :].rearrange("(n p) d -> p n d", p=P),
            )
            nc.sync.dma_start(
                out=htile[0:T - nmain * P, nmain * D:7 * D],
                in_=h_in[b, nmain * P:T, :],
            )
            u_sb = upool.tile([J, D], f32, tag="u")
            nc.sync.dma_start(out=u_sb[:, :], in_=u_in[b, :, :])

            # ---- u transposes, su row, s-matmul rhs ----
            uT = []
            for kc, (d0, kn) in enumerate(KCHUNKS):
                tp = ps_tp.tile([100, P], f32, tag="tp")
                nc.tensor.transpose(tp[:kn, :J], u_sb[:J, d0:d0 + kn], identity[:J, :J])
                uT_sb = upool.tile([100, J], f32, tag=f"uT{kc}")
                nc.scalar.copy(out=uT_sb[:kn, :], in_=tp[:kn, :J])
                uT.append(uT_sb)

            su_ps = ps_tp.tile([1, J], f32, tag="tp")
            for kc, (d0, kn) in enumerate(KCHUNKS):
                nc.tensor.matmul(su_ps[:1, :], lhsT=wcols[("wu", kc)][:, :],
                                 rhs=uT[kc][:KCHUNKS[kc][1], :],
                                 start=(kc == 0), stop=(kc == 1))
            su_sb = bsmall.tile([1, J + 1], f32, tag="su")
            nc.vector.memset(su_sb, 0.0)
            nc.scalar.copy(out=su_sb[:1, 0:J], in_=su_ps[:1, :])

            rhs_ext = []
            for kc, (d0, kn) in enumerate(KCHUNKS):
                re_ = upool.tile([100, J + 1], f32, tag=f"rhs{kc}")
                nc.vector.tensor_scalar_mul(
                    out=re_[:kn, 0:J], in0=uT[kc][:kn, :], scalar1=wcols[("whu", kc)][:, :])
                nc.gpsimd.tensor_copy(out=re_[:kn, J:J + 1], in_=wcols[("wh", kc)][:, :])
                rhs_ext.append(re_)

            # ---- h transpose: hT[kc] [100, 800] ----
            hT = [hTpool.tile([100, T], f32, tag=f"hT{kc}", name=f"hT{kc}")
                  for kc in range(2)]
            for c, (t0, rows) in enumerate(TCHUNKS):
                for kc, (d0, kn) in enumerate(KCHUNKS):
                    tp = ps_tp.tile([100, P], f32, tag="tp")
                    nc.tensor.transpose(
                        tp[:kn, :rows], htile[:rows, c * D + d0:c * D + d0 + kn],
                        identity[:rows, :rows])
                    nc.scalar.copy(out=hT[kc][:kn, t0:t0 + rows], in_=tp[:kn, :rows])

            e_all = bsmall.tile([P, 7], f32, tag="e_all")
            nc.gpsimd.memset(e_all, 0.0)
            y_ps = ps_acc.tile([1, D], f32, tag="acc")

            # ---- main chunk loop ----
            for c, (t0, rows) in enumerate(TCHUNKS):
                s_ps = ps_s.tile([P, J + 1], f32, tag="s")
                for kc, (d0, kn) in enumerate(KCHUNKS):
                    nc.tensor.matmul(s_ps[:rows, :], lhsT=hT[kc][:kn, t0:t0 + rows],
                                     rhs=rhs_ext[kc][:kn, :],
                                     start=(kc == 0), stop=False)
                nc.tensor.matmul(s_ps[:rows, :], lhsT=ones_row[:1, :rows],
                                 rhs=su_sb[:1, :], start=False, stop=True)

                m_sb = csmall.tile([P, 1], f32, tag="m")
                nc.vector.reduce_max(out=m_sb[:rows, :], in_=s_ps[:rows, 0:J], axis=AX.X)
                msh = csmall.tile([P, 1], f32, tag="msh")
                nc.vector.tensor_add(out=msh[:rows, :], in0=m_sb[:rows, :],
                                     in1=s_ps[:rows, J:J + 1])
                nc.scalar.activation(out=e_all[:rows, c:c + 1], in_=msh[:rows, :],
                                     func=AF.Exp)

                p_sb = csmall.tile([P, J], f32, tag="p")
                rsum = csmall.tile([P, 1], f32, tag="rsum")
                nc.scalar.activation(out=p_sb[:rows, :], in_=s_ps[:rows, 0:J],
                                     func=AF.Exp, accum_out=rsum[:rows, :])
                rcp = csmall.tile([P, 1], f32, tag="rcp")
                nc.vector.reciprocal(out=rcp[:rows, :], in_=rsum[:rows, :])

                tp2 = ps_tp.tile([100, P], f32, tag="tp")
                nc.tensor.transpose(tp2[:J, :rows], p_sb[:rows, :J],
                                    identity[:rows, :rows])
                pT_sb = csmall.tile([J, P], f32, tag="pT")
                nc.scalar.copy(out=pT_sb[:J, :rows], in_=tp2[:J, :rows])

                cps = ps_c2q.tile([P, D], f32, tag="c2q")
                nc.tensor.matmul(cps[:rows, :], lhsT=pT_sb[:J, :rows],
                                 rhs=u_sb[:J, :], start=True, stop=True)

                nc.tensor.matmul(y_ps[:1, :], lhsT=e_all[:rows, c:c + 1],
                                 rhs=htile[:rows, c * D:(c + 1) * D],
                                 start=(c == 0), stop=(c == 6))

                g600 = gpool.tile([P, 3 * D], f32, tag="g600")
                nc.gpsimd.tensor_copy(out=g600[:rows, 0:D],
                                      in_=htile[:rows, c * D:(c + 1) * D])
                nc.vector.tensor_scalar_mul(out=g600[:rows, D:2 * D],
                                            in0=cps[:rows, :], scalar1=rcp[:rows, :])
                nc.vector.tensor_mul(out=g600[:rows, 2 * D:3 * D],
                                     in0=htile[:rows, c * D:(c + 1) * D],
                                     in1=g600[:rows, D:2 * D])
                nc.sync.dma_start(out=g_out[b, t0:t0 + rows, 0:3 * D],
                                  in_=g600[:rows, :])

            # ---- batch tail: q2c ----
            S_ps = ps_acc.tile([1, 7], f32, tag="acc")
            nc.tensor.matmul(S_ps[:1, :], lhsT=ones_col[:P, :1], rhs=e_all[:, :],
                             start=True, stop=True)
            Ssum = bsmall.tile([1, 1], f32, tag="Ssum")
            nc.vector.reduce_sum(out=Ssum[:1, :], in_=S_ps[:1, :], axis=AX.X)
            Sinv = bsmall.tile([1, 1], f32, tag="Sinv")
            nc.vector.reciprocal(out=Sinv[:1, :], in_=Ssum[:1, :])
            q2c_sb = bsmall.tile([1, D], f32, tag="q2c")
            nc.vector.tensor_scalar_mul(out=q2c_sb[:1, :], in0=y_ps[:1, :],
                                        scalar1=Sinv[:1, :])
            q2cb_ps = ps_acc.tile([P, D], f32, tag="acc")
            nc.tensor.matmul(q2cb_ps[:, :], lhsT=ones_row[:1, :], rhs=q2c_sb[:1, :],
                             start=True, stop=True)
            q2cb_sb = bsmall.tile([P, D], f32, tag="q2cb")
            nc.scalar.copy(out=q2cb_sb[:, :], in_=q2cb_ps[:, :])

            for c, (t0, rows) in enumerate(TCHUNKS):
                hq = hqpool.tile([P, D], f32, tag="hq")
                nc.gpsimd.tensor_mul(out=hq[:rows, :],
                                     in0=htile[:rows, c * D:(c + 1) * D],
                                     in1=q2cb_sb[:rows, :])
                nc.sync.dma_start(out=g_out[b, t0:t0 + rows, 3 * D:4 * D],
                                  in_=hq[:rows, :])

    return nc


def kernel(h, u, w_h, b_h, w_u, b_u, w_hu, b_hu):
    from concourse.bass_utils import run_bass_kernel_spmd

    if "nc" not in _cache:
        nc = _build()
        _split_multi_waits(nc)
        _cache["nc"] = nc
    nc = _cache["nc"]

    h = np.ascontiguousarray(h, dtype=np.float32)
    u = np.ascontiguousarray(u, dtype=np.float32)
    w_h = np.ascontiguousarray(w_h, dtype=np.float32)
    w_u = np.ascontiguousarray(w_u, dtype=np.float32)
    w_hu = np.ascontiguousarray(w_hu, dtype=np.float32)

    core_ids = list(range(NCORES))
    in_maps = []
    for i in core_ids:
        in_maps.append({
            "h": h[i * BC:(i + 1) * BC],
            "u": u[i * BC:(i + 1) * BC],
            "w_h": w_h,
            "w_u": w_u,
            "w_hu": w_hu,
        })
    res = run_bass_kernel_spmd(nc, in_maps, core_ids)
    _cache["last_results"] = res
    return np.concatenate([res.results[i]["g"] for i in core_ids], axis=0)


# revision 24
# speedup vs baseline: 5.3248x; 5.3248x over previous
"""BiDAF attention-flow layer on 8 Trainium2 NeuronCores.

Data-parallel over batch: each core processes B/8 = 8 batches.

Math (per batch b):
  s[t,j] = h[t]·w_h + u[j]·w_u + (h[t]*w_hu)·u[j] + const
  a      = softmax_j(s)            -> only needs  sj = shu + su  (row consts cancel)
  c2q    = a @ u
  bt     = softmax_t(max_j s)      -> needs  m + sh  where m = max_j(sj)
  q2c    = bt @ h
  g      = [h | c2q | h*c2q | h*q2c]

The rank-1 bias terms b_h/b_u/b_hu shift every s[t,j] equally and cancel in
both softmaxes, so they are accepted but unused.

Layout per batch (core-local):
  htile [128, 7*200]   h rows chunked by 128 (chunk c at cols c*200..)
  hT    2x [100, 800]  h transposed (PE transpose), K-chunks of D
  s_ps  [128, 51]      cols 0:50 = shu+su (via K=1 ones-matmul), col 50 = sh
  softmax on free dim; p transposed back (PE) for the c2q matmul;
  y[1,200] = sum_t e_t h_t accumulated in PSUM over chunks; q2c = y/sum(e).
"""
import sys

if '/opt/trn_rl_repo' not in sys.path:
    sys.path.insert(0, '/opt/trn_rl_repo')

import numpy as np

B, T, J, D = 64, 800, 50, 200
NCORES = 8
BC = B // NCORES            # batches per core
P = 128
TCHUNKS = [(c * P, min(P, T - c * P)) for c in range((T + P - 1) // P)]
KCHUNKS = [(0, 100), (100, 100)]
NPAD = 256

_cache = {}
F32R = False  # f32r c2q is ~8us faster but 50x less accurate; keep exact


def _split_multi_waits(nc, max_waits=1):
    """This walrus build accepts at most one sync-wait per instruction.
    For any instruction carrying more, move the extra waits onto pure-wait
    EventSemaphore carriers inserted just before it on the same engine —
    the sequencer dispatches in order, so the blocking behavior is
    identical."""
    from concourse import mybir
    import bass_rust
    n = 0
    for f in nc.m.functions:
        for blk in f.blocks:
            insts = blk.instructions
            i = 0
            while i < len(insts):
                inst = insts[i]
                si = inst.sync_info
                if si is not None and len(si.on_wait) > max_waits:
                    waits = list(si.on_wait)
                    keep = waits[-max_waits:]
                    new = []
                    for w in waits[:-max_waits]:
                        d = mybir.InstEventSemaphore(
                            name=f"{inst.name}-sw{n}", ins=[], outs=[])
                        n += 1
                        d.engine = inst.engine
                        d.sync_info = bass_rust.SyncInfo(on_wait=[w], on_update=[])
                        new.append(d)
                    inst.sync_info = bass_rust.SyncInfo(
                        on_wait=keep, on_update=list(si.on_update))
                    for j, d in enumerate(new):
                        insts.insert(i + j, d)
                    i += len(new)
                i += 1
    return n


def _build(reps=1):
    import concourse.bass as bass
    import concourse.tile as tile
    from concourse import mybir, masks
    from contextlib import ExitStack

    f32 = mybir.dt.float32
    f32r = mybir.dt.float32r
    AF = mybir.ActivationFunctionType
    AX = mybir.AxisListType

    nc = bass.Bass()
    h_in = nc.declare_dram_parameter("h", [BC, T, D], f32, isOutput=False)
    u_in = nc.declare_dram_parameter("u", [BC, J, D], f32, isOutput=False)
    wh_in = nc.declare_dram_parameter("w_h", [D], f32, isOutput=False)
    wu_in = nc.declare_dram_parameter("w_u", [D], f32, isOutput=False)
    whu_in = nc.declare_dram_parameter("w_hu", [D], f32, isOutput=False)
    g_out = nc.declare_dram_parameter("g", [BC, T, 4 * D], f32, isOutput=True)

    with tile.TileContext(nc) as tc, ExitStack() as ctx:
        singles = ctx.enter_context(tc.tile_pool(name="singles", bufs=1))
        hpool = ctx.enter_context(tc.tile_pool(name="hpool", bufs=4))
        hTpool = ctx.enter_context(tc.tile_pool(name="hTpool", bufs=3))
        upool = ctx.enter_context(tc.tile_pool(name="upool", bufs=3))
        bsmall = ctx.enter_context(tc.tile_pool(name="bsmall", bufs=4))
        csmall = ctx.enter_context(tc.tile_pool(name="csmall", bufs=6))
        gpool = ctx.enter_context(tc.tile_pool(name="gpool", bufs=8))
        hqpool = ctx.enter_context(tc.tile_pool(name="hqpool", bufs=3))
        ps_s = ctx.enter_context(
            tc.tile_pool(name="ps_s", bufs=2, space=bass.MemorySpace.PSUM))
        ps_tp = ctx.enter_context(
            tc.tile_pool(name="ps_tp", bufs=2, space=bass.MemorySpace.PSUM))
        ps_c2q = ctx.enter_context(
            tc.tile_pool(name="ps_c2q", bufs=2, space=bass.MemorySpace.PSUM))
        ps_acc = ctx.enter_context(
            tc.tile_pool(name="ps_acc", bufs=2, space=bass.MemorySpace.PSUM))

        # ---- once-per-core constants ----
        identity = singles.tile([P, P], f32)
        masks.make_identity(nc, identity[:])
        ones_row = singles.tile([1, P], f32)
        nc.vector.memset(ones_row, 1.0)
        ones_col = singles.tile([P, 1], f32)
        nc.vector.memset(ones_col, 1.0)

        wcols = {}
        for wname, wsrc in (("wh", wh_in), ("wu", wu_in), ("whu", whu_in)):
            t_ = singles.tile([100, 2], f32, tag=wname, name=wname)
            nc.sync.dma_start(out=t_[:, :], in_=wsrc.rearrange("(k p) -> p k", p=100))
            for kc in range(2):
                wcols[(wname, kc)] = t_[:, kc:kc + 1]

        def batch_body(b):
            # ---- load u first (small; unblocks u-prep), then h ----
            u_sb = upool.tile([J, NPAD if F32R else D], f32, tag="u", name="u_sb")
            if F32R:
                nc.vector.memset(u_sb[:, D:NPAD], 0.0)
            nc.sync.dma_start(out=u_sb[:, 0:D], in_=u_in[b, :, :])
            if F32R:
                u_r = upool.tile([J, NPAD], f32, tag="u_r", name="u_r")
                nc.scalar.copy(out=u_r[:, :].bitcast(f32r), in_=u_sb[:, 0:NPAD])
            hcols = 7 * D + (NPAD - D if F32R else 0)
            htile = hpool.tile([P, hcols], f32, tag="h", name="htile")
            if F32R:
                nc.vector.memset(htile[:, 7 * D:hcols], 0.0)
            nmain = 6
            nc.sync.dma_start(
                out=htile[:, 0:nmain * D].rearrange("p (n d) -> p n d", d=D),
                in_=h_in[b, 0:nmain * P, :].rearrange("(n p) d -> p n d", p=P),
            )
            if F32R:
                # rows past the ragged tail are read by the padded y-matmul;
                # zero the whole block, the tail DMA then overwrites rows 0:32
                nc.vector.memset(htile[:, nmain * D:7 * D], 0.0)
            nc.sync.dma_start(
                out=htile[0:T - nmain * P, nmain * D:7 * D],
                in_=h_in[b, nmain * P:T, :],
            )

            # ---- u transposes, su row, s-matmul rhs ----
            uT = []
            for kc, (d0, kn) in enumerate(KCHUNKS):
                tp = ps_tp.tile([100, P], f32, tag="tp")
                nc.tensor.transpose(tp[:kn, :J], u_sb[:J, d0:d0 + kn], identity[:J, :J])
                uT_sb = upool.tile([100, J], f32, tag=f"uT{kc}")
                nc.scalar.copy(out=uT_sb[:kn, :], in_=tp[:kn, :J])
                uT.append(uT_sb)

            su_ps = ps_tp.tile([1, J], f32, tag="tp")
            for kc, (d0, kn) in enumerate(KCHUNKS):
                nc.tensor.matmul(su_ps[:1, :], lhsT=wcols[("wu", kc)],
                                 rhs=uT[kc][:KCHUNKS[kc][1], :],
                                 start=(kc == 0), stop=(kc == 1))
            su_sb = bsmall.tile([1, J + 1], f32, tag="su")
            nc.vector.memset(su_sb, 0.0)
            nc.scalar.copy(out=su_sb[:1, 0:J], in_=su_ps[:1, :])

            rhs_ext = []
            for kc, (d0, kn) in enumerate(KCHUNKS):
                re_ = upool.tile([100, J + 1], f32, tag=f"rhs{kc}", name=f"rhs{kc}")
                nc.vector.tensor_scalar_mul(
                    out=re_[:kn, 0:J], in0=uT[kc][:kn, :], scalar1=wcols[("whu", kc)])
                nc.gpsimd.tensor_copy(out=re_[:kn, J:J + 1], in_=wcols[("wh", kc)])
                rhs_ext.append(re_)

            # ---- h transpose: hT [101, 2*800]; row 100 = ones (su path) ----
            hT = hTpool.tile([100, 2 * T], f32, tag="hT")
            for c, (t0, rows) in enumerate(TCHUNKS):
                tp = ps_tp.tile([100, 2 * P], f32, tag="tp")
                for kc, (d0, kn) in enumerate(KCHUNKS):
                    nc.tensor.matmul(
                        tp[:kn, kc * P:kc * P + rows],
                        lhsT=htile[:rows, c * D + d0:c * D + d0 + kn],
                        rhs=identity[:rows, :rows], is_transpose=True,
                        skip_group_check=True)
                nc.scalar.copy(
                    out=hT[:100, :].rearrange("p (k t) -> p k t", k=2)[:, :, t0:t0 + rows],
                    in_=tp[:100, :].rearrange("p (k c) -> p k c", k=2)[:, :, :rows])

            e_all = bsmall.tile([P, 7], f32, tag="e_all")
            nc.gpsimd.memset(e_all, 0.0)
            y_ps = ps_acc.tile([1, D], f32, tag="acc", name="y_ps")
            gtiles = []

            # ---- main chunk loop ----
            for c, (t0, rows) in enumerate(TCHUNKS):
                s_ps = ps_s.tile([P, J + 1], f32, tag="s")
                nc.tensor.matmul(s_ps[:rows, :], lhsT=hT[0:100, t0:t0 + rows],
                                 rhs=rhs_ext[0][:100, :], start=True, stop=False)
                nc.tensor.matmul(s_ps[:rows, :],
                                 lhsT=hT[0:100, T + t0:T + t0 + rows],
                                 rhs=rhs_ext[1][:100, :], start=False, stop=False)
                nc.tensor.matmul(s_ps[:rows, :], lhsT=ones_row[:1, :rows],
                                 rhs=su_sb[:1, :], start=False, stop=True)

                m_sb = csmall.tile([P, 1], f32, tag="m")
                nc.vector.reduce_max(out=m_sb[:rows, :], in_=s_ps[:rows, 0:J], axis=AX.X)
                msh = csmall.tile([P, 1], f32, tag="msh")
                nc.vector.tensor_add(out=msh[:rows, :], in0=m_sb[:rows, :],
                                     in1=s_ps[:rows, J:J + 1])
                nc.scalar.activation(out=e_all[:rows, c:c + 1], in_=msh[:rows, :],
                                     func=AF.Exp)

                p_sb = csmall.tile([P, J], f32, tag="p")
                rsum = csmall.tile([P, 1], f32, tag="rsum")
                nc.scalar.activation(out=p_sb[:rows, :], in_=s_ps[:rows, 0:J],
                                     func=AF.Exp, accum_out=rsum[:rows, :])
                rcp = csmall.tile([P, 1], f32, tag="rcp")
                nc.vector.reciprocal(out=rcp[:rows, :], in_=rsum[:rows, :])

                tp2 = ps_tp.tile([100, P], f32, tag="tp")
                nc.tensor.transpose(tp2[:J, :rows], p_sb[:rows, :J],
                                    identity[:rows, :rows])
                pT_sb = csmall.tile([J, P], f32, tag="pT")
                nc.scalar.copy(out=pT_sb[:J, :rows].bitcast(f32r) if F32R
                               else pT_sb[:J, :rows], in_=tp2[:J, :rows])

                if F32R:
                    cps = ps_c2q.tile([P, NPAD], f32, tag="c2q")
                    nc.tensor.matmul(cps[:rows, 0:NPAD],
                                     lhsT=pT_sb[:J, :rows].bitcast(f32r),
                                     rhs=u_r[:J, 0:NPAD].bitcast(f32r),
                                     start=True, stop=True)
                else:
                    cps = ps_c2q.tile([P, D], f32, tag="c2q")
                    nc.tensor.matmul(cps[:rows, :], lhsT=pT_sb[:J, :rows],
                                     rhs=u_sb[:J, 0:D], start=True, stop=True)
                nc.tensor.matmul(y_ps[:1, :], lhsT=e_all[:rows, c:c + 1],
                                 rhs=htile[:rows, c * D:(c + 1) * D],
                                 start=(c == 0), stop=(c == 6))

                if c % 2 == 0:
                    gt = gpool.tile([P, 6 * D], f32, tag="g", name="gt")
                    gtiles.append(gt)
                else:
                    gt = gtiles[-1]
                go = (c % 2) * 3 * D
                nc.gpsimd.tensor_copy(out=gt[:rows, go:go + D],
                                      in_=htile[:rows, c * D:(c + 1) * D])
                nc.vector.tensor_scalar_mul(out=gt[:rows, go + D:go + 2 * D],
                                            in0=cps[:rows, 0:D], scalar1=rcp[:rows, :])
                nc.vector.tensor_mul(out=gt[:rows, go + 2 * D:go + 3 * D],
                                     in0=htile[:rows, c * D:(c + 1) * D],
                                     in1=gt[:rows, go + D:go + 2 * D])
                if c in (1, 3, 5):
                    # one DMA covers two 128-row chunks x cols 0:600
                    nc.sync.dma_start(
                        out=g_out[b, t0 - P:t0 + P, 0:3 * D].rearrange(
                            "(k p) x -> p k x", p=P),
                        in_=gt[:, :].rearrange("p (k x) -> p k x", k=2))
                elif c == 6:
                    nc.sync.dma_start(out=g_out[b, t0:t0 + rows, 0:3 * D],
                                      in_=gt[:rows, 0:3 * D])

            # ---- batch tail: q2c ----
            S_ps = ps_acc.tile([1, 7], f32, tag="acc")
            nc.tensor.matmul(S_ps[:1, :], lhsT=ones_col[:P, :1], rhs=e_all[:, :],
                             start=True, stop=True)
            Ssum = bsmall.tile([1, 1], f32, tag="Ssum")
            nc.vector.reduce_sum(out=Ssum[:1, :], in_=S_ps[:1, :], axis=AX.X)
            Sinv = bsmall.tile([1, 1], f32, tag="Sinv")
            nc.vector.reciprocal(out=Sinv[:1, :], in_=Ssum[:1, :])
            q2c_sb = bsmall.tile([1, D], f32, tag="q2c")
            nc.vector.tensor_scalar_mul(out=q2c_sb[:1, :], in0=y_ps[:1, 0:D],
                                        scalar1=Sinv[:1, :])
            q2cb_ps = ps_acc.tile([P, D], f32, tag="acc")
            nc.tensor.matmul(q2cb_ps[:, :], lhsT=ones_row[:1, :], rhs=q2c_sb[:1, :],
                             start=True, stop=True)
            q2cb_sb = bsmall.tile([P, D], f32, tag="q2cb")
            nc.scalar.copy(out=q2cb_sb[:, :], in_=q2cb_ps[:, :])

            hq_all = hqpool.tile([P, 7 * D], f32, tag="hq")
            q2cb_b6 = bass.AP(tensor=q2cb_sb.tensor, offset=q2cb_sb.offset,
                              ap=[q2cb_sb.ap[0], [0, 6], q2cb_sb.ap[1]])
            nc.gpsimd.tensor_mul(
                out=hq_all[:, 0:6 * D].rearrange("p (n d) -> p n d", d=D),
                in0=htile[:, 0:6 * D].rearrange("p (n d) -> p n d", d=D),
                in1=q2cb_b6)
            nc.gpsimd.tensor_mul(out=hq_all[0:T - 6 * P, 6 * D:7 * D],
                                 in0=htile[0:T - 6 * P, 6 * D:7 * D],
                                 in1=q2cb_sb[0:T - 6 * P, :])
            nc.sync.dma_start(
                out=g_out[b, 0:6 * P, 3 * D:4 * D].rearrange("(n p) d -> p n d", p=P),
                in_=hq_all[:, 0:6 * D].rearrange("p (n d) -> p n d", d=D))
            nc.sync.dma_start(out=g_out[b, 6 * P:T, 3 * D:4 * D],
                              in_=hq_all[0:T - 6 * P, 6 * D:7 * D])

        if reps == 1:
            for b in range(BC):
                batch_body(b)
        else:
            with tc.For_i(0, reps, 1):
                for b in range(BC):
                    batch_body(b)

    return nc


def kernel(h, u, w_h, b_h, w_u, b_u, w_hu, b_hu):
    from concourse.bass_utils import run_bass_kernel_spmd

    if "nc" not in _cache:
        nc = _build()
        _split_multi_waits(nc)
        _cache["nc"] = nc
    nc = _cache["nc"]

    h = np.ascontiguousarray(h, dtype=np.float32)
    u = np.ascontiguousarray(u, dtype=np.float32)
    w_h = np.ascontiguousarray(w_h, dtype=np.float32)
    w_u = np.ascontiguousarray(w_u, dtype=np.float32)
    w_hu = np.ascontiguousarray(w_hu, dtype=np.float32)

    core_ids = list(range(NCORES))
    in_maps = []
    for i in core_ids:
        in_maps.append({
            "h": h[i * BC:(i + 1) * BC],
            "u": u[i * BC:(i + 1) * BC],
            "w_h": w_h,
            "w_u": w_u,
            "w_hu": w_hu,
        })
    res = run_bass_kernel_spmd(nc, in_maps, core_ids)
    _cache["last_results"] = res
    return np.concatenate([res.results[i]["g"] for i in core_ids], axis=0)


# revision 27
# speedup vs baseline: 6.0180x; 1.1302x over previous
"""BiDAF attention-flow layer on 8 Trainium2 NeuronCores.

Data-parallel over batch: each core processes B/8 = 8 batches.

Math (per batch b):
  s[t,j] = h[t]·w_h + u[j]·w_u + (h[t]*w_hu)·u[j] + const
  a      = softmax_j(s)            -> only needs  sj = shu + su  (row consts cancel)
  c2q    = a @ u
  bt     = softmax_t(max_j s)      -> needs  m + sh  where m = max_j(sj)
  q2c    = bt @ h
  g      = [h | c2q | h*c2q | h*q2c]

The rank-1 bias terms b_h/b_u/b_hu shift every s[t,j] equally and cancel in
both softmaxes, so they are accepted but unused.

Layout per batch (core-local):
  htile [128, 7*200]   h rows chunked by 128 (chunk c at cols c*200..)
  hT    2x [100, 800]  h transposed (PE transpose), K-chunks of D
  s_ps  [128, 51]      cols 0:50 = shu+su (via K=1 ones-matmul), col 50 = sh
  softmax on free dim; p transposed back (PE) for the c2q matmul;
  y[1,200] = sum_t e_t h_t accumulated in PSUM over chunks; q2c = y/sum(e).
"""
import sys

if '/opt/trn_rl_repo' not in sys.path:
    sys.path.insert(0, '/opt/trn_rl_repo')

import numpy as np

B, T, J, D = 64, 800, 50, 200
NCORES = 8
BC = B // NCORES            # batches per core
P = 128
TCHUNKS = [(c * P, min(P, T - c * P)) for c in range((T + P - 1) // P)]
KCHUNKS = [(0, 100), (100, 100)]
NPAD = 256

_cache = {}
F32R = False  # f32r c2q is ~8us faster but 50x less accurate; keep exact


def _split_multi_waits(nc, max_waits=1):
    """This walrus build accepts at most one sync-wait per instruction.
    For any instruction carrying more, move the extra waits onto pure-wait
    EventSemaphore carriers inserted just before it on the same engine —
    the sequencer dispatches in order, so the blocking behavior is
    identical."""
    from concourse import mybir
    import bass_rust
    n = 0
    for f in nc.m.functions:
        for blk in f.blocks:
            insts = blk.instructions
            i = 0
            while i < len(insts):
                inst = insts[i]
                si = inst.sync_info
                if si is not None and len(si.on_wait) > max_waits:
                    waits = list(si.on_wait)
                    keep = waits[-max_waits:]
                    new = []
                    for w in waits[:-max_waits]:
                        d = mybir.InstEventSemaphore(
                            name=f"{inst.name}-sw{n}", ins=[], outs=[])
                        n += 1
                        d.engine = inst.engine
                        d.sync_info = bass_rust.SyncInfo(on_wait=[w], on_update=[])
                        new.append(d)
                    inst.sync_info = bass_rust.SyncInfo(
                        on_wait=keep, on_update=list(si.on_update))
                    for j, d in enumerate(new):
                        insts.insert(i + j, d)
                    i += len(new)
                i += 1
    return n


def _build(reps=1):
    import concourse.bass as bass
    import concourse.tile as tile
    from concourse import mybir, masks
    from contextlib import ExitStack

    f32 = mybir.dt.float32
    f32r = mybir.dt.float32r
    AF = mybir.ActivationFunctionType
    AX = mybir.AxisListType

    nc = bass.Bass()
    h_in = nc.declare_dram_parameter("h", [BC, T, D], f32, isOutput=False)
    u_in = nc.declare_dram_parameter("u", [BC, J, D], f32, isOutput=False)
    wh_in = nc.declare_dram_parameter("w_h", [D], f32, isOutput=False)
    wu_in = nc.declare_dram_parameter("w_u", [D], f32, isOutput=False)
    whu_in = nc.declare_dram_parameter("w_hu", [D], f32, isOutput=False)
    g_out = nc.declare_dram_parameter("g", [BC, T, 4 * D], f32, isOutput=True)

    with tile.TileContext(nc) as tc, ExitStack() as ctx:
        singles = ctx.enter_context(tc.tile_pool(name="singles", bufs=1))
        hpool = ctx.enter_context(tc.tile_pool(name="hpool", bufs=4))
        hTpool = ctx.enter_context(tc.tile_pool(name="hTpool", bufs=3))
        upool = ctx.enter_context(tc.tile_pool(name="upool", bufs=3))
        bsmall = ctx.enter_context(tc.tile_pool(name="bsmall", bufs=4))
        csmall = ctx.enter_context(tc.tile_pool(name="csmall", bufs=6))
        gpool = ctx.enter_context(tc.tile_pool(name="gpool", bufs=8))
        hqpool = ctx.enter_context(tc.tile_pool(name="hqpool", bufs=3))
        ps_s = ctx.enter_context(
            tc.tile_pool(name="ps_s", bufs=2, space=bass.MemorySpace.PSUM))
        ps_tp = ctx.enter_context(
            tc.tile_pool(name="ps_tp", bufs=2, space=bass.MemorySpace.PSUM))
        ps_c2q = ctx.enter_context(
            tc.tile_pool(name="ps_c2q", bufs=2, space=bass.MemorySpace.PSUM))
        ps_acc = ctx.enter_context(
            tc.tile_pool(name="ps_acc", bufs=2, space=bass.MemorySpace.PSUM))

        # ---- once-per-core constants ----
        identity = singles.tile([P, P], f32)
        masks.make_identity(nc, identity[:])
        ones_row = singles.tile([1, P], f32)
        nc.vector.memset(ones_row, 1.0)
        ones_col = singles.tile([P, 1], f32)
        nc.vector.memset(ones_col, 1.0)

        wcols = {}
        for wname, wsrc in (("wh", wh_in), ("wu", wu_in), ("whu", whu_in)):
            t_ = singles.tile([100, 2], f32, tag=wname, name=wname)
            nc.sync.dma_start(out=t_[:, :], in_=wsrc.rearrange("(k p) -> p k", p=100))
            for kc in range(2):
                wcols[(wname, kc)] = t_[:, kc:kc + 1]

        def batch_body(b):
            # ---- load u first (small; unblocks u-prep), then h ----
            u_sb = upool.tile([J, NPAD if F32R else D], f32, tag="u", name="u_sb")
            if F32R:
                nc.vector.memset(u_sb[:, D:NPAD], 0.0)
            nc.sync.dma_start(out=u_sb[:, 0:D], in_=u_in[b, :, :])
            if F32R:
                u_r = upool.tile([J, NPAD], f32, tag="u_r", name="u_r")
                nc.scalar.copy(out=u_r[:, :].bitcast(f32r), in_=u_sb[:, 0:NPAD])
            hcols = 7 * D + (NPAD - D if F32R else 0)
            htile = hpool.tile([P, hcols], f32, tag="h", name="htile")
            if F32R:
                nc.vector.memset(htile[:, 7 * D:hcols], 0.0)
            nmain = 6
            nc.sync.dma_start(
                out=htile[:, 0:nmain * D].rearrange("p (n d) -> p n d", d=D),
                in_=h_in[b, 0:nmain * P, :].rearrange("(n p) d -> p n d", p=P),
            )
            if F32R:
                # rows past the ragged tail are read by the padded y-matmul;
                # zero the whole block, the tail DMA then overwrites rows 0:32
                nc.vector.memset(htile[:, nmain * D:7 * D], 0.0)
            nc.sync.dma_start(
                out=htile[0:T - nmain * P, nmain * D:7 * D],
                in_=h_in[b, nmain * P:T, :],
            )

            # ---- u transposes, su row, s-matmul rhs ----
            uT = []
            for kc, (d0, kn) in enumerate(KCHUNKS):
                tp = ps_tp.tile([100, P], f32, tag="tp")
                nc.tensor.transpose(tp[:kn, :J], u_sb[:J, d0:d0 + kn], identity[:J, :J])
                uT_sb = upool.tile([100, J], f32, tag=f"uT{kc}")
                nc.scalar.copy(out=uT_sb[:kn, :], in_=tp[:kn, :J])
                uT.append(uT_sb)

            su_ps = ps_acc.tile([1, J], f32, tag="acc", name="su_ps")
            for kc, (d0, kn) in enumerate(KCHUNKS):
                nc.tensor.matmul(su_ps[:1, :], lhsT=wcols[("wu", kc)],
                                 rhs=uT[kc][:KCHUNKS[kc][1], :],
                                 start=(kc == 0), stop=(kc == 1))
            su_sb = bsmall.tile([1, J + 1], f32, tag="su")
            nc.vector.memset(su_sb, 0.0)
            nc.scalar.copy(out=su_sb[:1, 0:J], in_=su_ps[:1, :])

            rhs_ext = []
            for kc, (d0, kn) in enumerate(KCHUNKS):
                re_ = upool.tile([100, J + 1], f32, tag=f"rhs{kc}", name=f"rhs{kc}")
                nc.vector.tensor_scalar_mul(
                    out=re_[:kn, 0:J], in0=uT[kc][:kn, :], scalar1=wcols[("whu", kc)])
                nc.gpsimd.tensor_copy(out=re_[:kn, J:J + 1], in_=wcols[("wh", kc)])
                rhs_ext.append(re_)

            # ---- h transpose: hT [101, 2*800]; row 100 = ones (su path) ----
            hT = hTpool.tile([100, 2 * T], f32, tag="hT")
            for c, (t0, rows) in enumerate(TCHUNKS):
                tp = ps_tp.tile([100, 2 * P], f32, tag="tp")
                for kc, (d0, kn) in enumerate(KCHUNKS):
                    nc.tensor.matmul(
                        tp[:kn, kc * P:kc * P + rows],
                        lhsT=htile[:rows, c * D + d0:c * D + d0 + kn],
                        rhs=identity[:rows, :rows], is_transpose=True,
                        skip_group_check=True)
                nc.scalar.copy(
                    out=hT[:100, :].rearrange("p (k t) -> p k t", k=2)[:, :, t0:t0 + rows],
                    in_=tp[:100, :].rearrange("p (k c) -> p k c", k=2)[:, :, :rows])

            e_all = bsmall.tile([P, 7], f32, tag="e_all")
            nc.gpsimd.memset(e_all, 0.0)
            y_ps = ps_acc.tile([1, D], f32, tag="acc", name="y_ps")
            gtiles = []

            # ---- main chunk loop ----
            for c, (t0, rows) in enumerate(TCHUNKS):
                s_ps = ps_s.tile([P, J + 1], f32, tag="s")
                nc.tensor.matmul(s_ps[:rows, :], lhsT=hT[0:100, t0:t0 + rows],
                                 rhs=rhs_ext[0][:100, :], start=True, stop=False)
                nc.tensor.matmul(s_ps[:rows, :],
                                 lhsT=hT[0:100, T + t0:T + t0 + rows],
                                 rhs=rhs_ext[1][:100, :], start=False, stop=False)
                nc.tensor.matmul(s_ps[:rows, :], lhsT=ones_row[:1, :rows],
                                 rhs=su_sb[:1, :], start=False, stop=True)

                m_sb = csmall.tile([P, 1], f32, tag="m")
                nc.vector.reduce_max(out=m_sb[:rows, :], in_=s_ps[:rows, 0:J], axis=AX.X)
                msh = csmall.tile([P, 1], f32, tag="msh")
                nc.vector.tensor_add(out=msh[:rows, :], in0=m_sb[:rows, :],
                                     in1=s_ps[:rows, J:J + 1])
                nc.scalar.activation(out=e_all[:rows, c:c + 1], in_=msh[:rows, :],
                                     func=AF.Exp)

                p_sb = csmall.tile([P, J], f32, tag="p")
                rsum = csmall.tile([P, 1], f32, tag="rsum")
                nc.scalar.activation(out=p_sb[:rows, :], in_=s_ps[:rows, 0:J],
                                     func=AF.Exp, accum_out=rsum[:rows, :])
                rcp = csmall.tile([P, 1], f32, tag="rcp")
                nc.vector.reciprocal(out=rcp[:rows, :], in_=rsum[:rows, :])

                tp2 = ps_tp.tile([100, P], f32, tag="tp")
                nc.tensor.transpose(tp2[:J, :rows], p_sb[:rows, :J],
                                    identity[:rows, :rows])
                pT_sb = csmall.tile([J, P], f32, tag="pT")
                if F32R:
                    nc.scalar.copy(out=pT_sb[:J, :rows].bitcast(f32r),
                                   in_=tp2[:J, :rows])
                else:
                    nc.vector.tensor_copy(out=pT_sb[:J, :rows], in_=tp2[:J, :rows])

                if F32R:
                    cps = ps_c2q.tile([P, NPAD], f32, tag="c2q")
                    nc.tensor.matmul(cps[:rows, 0:NPAD],
                                     lhsT=pT_sb[:J, :rows].bitcast(f32r),
                                     rhs=u_r[:J, 0:NPAD].bitcast(f32r),
                                     start=True, stop=True)
                else:
                    cps = ps_c2q.tile([P, D], f32, tag="c2q")
                    nc.tensor.matmul(cps[:rows, :], lhsT=pT_sb[:J, :rows],
                                     rhs=u_sb[:J, 0:D], start=True, stop=True)
                nc.tensor.matmul(y_ps[:1, :], lhsT=e_all[:rows, c:c + 1],
                                 rhs=htile[:rows, c * D:(c + 1) * D],
                                 start=(c == 0), stop=(c == 6))

                if c % 2 == 0:
                    gt = gpool.tile([P, 6 * D], f32, tag="g", name="gt")
                    gtiles.append(gt)
                else:
                    gt = gtiles[-1]
                go = (c % 2) * 3 * D
                nc.gpsimd.tensor_copy(out=gt[:rows, go:go + D],
                                      in_=htile[:rows, c * D:(c + 1) * D])
                nc.vector.tensor_scalar_mul(out=gt[:rows, go + D:go + 2 * D],
                                            in0=cps[:rows, 0:D], scalar1=rcp[:rows, :])
                nc.gpsimd.tensor_mul(out=gt[:rows, go + 2 * D:go + 3 * D],
                                     in0=htile[:rows, c * D:(c + 1) * D],
                                     in1=gt[:rows, go + D:go + 2 * D])
                if c in (1, 3, 5):
                    # one DMA covers two 128-row chunks x cols 0:600
                    nc.sync.dma_start(
                        out=g_out[b, t0 - P:t0 + P, 0:3 * D].rearrange(
                            "(k p) x -> p k x", p=P),
                        in_=gt[:, :].rearrange("p (k x) -> p k x", k=2))
                elif c == 6:
                    nc.sync.dma_start(out=g_out[b, t0:t0 + rows, 0:3 * D],
                                      in_=gt[:rows, 0:3 * D])

            # ---- batch tail: q2c ----
            S_ps = ps_acc.tile([1, 7], f32, tag="acc")
            nc.tensor.matmul(S_ps[:1, :], lhsT=ones_col[:P, :1], rhs=e_all[:, :],
                             start=True, stop=True)
            Ssum = bsmall.tile([1, 1], f32, tag="Ssum")
            nc.vector.reduce_sum(out=Ssum[:1, :], in_=S_ps[:1, :], axis=AX.X)
            Sinv = bsmall.tile([1, 1], f32, tag="Sinv")
            nc.vector.reciprocal(out=Sinv[:1, :], in_=Ssum[:1, :])
            q2c_sb = bsmall.tile([1, D], f32, tag="q2c")
            nc.vector.tensor_scalar_mul(out=q2c_sb[:1, :], in0=y_ps[:1, 0:D],
                                        scalar1=Sinv[:1, :])
            q2cb_ps = ps_acc.tile([P, D], f32, tag="acc")
            nc.tensor.matmul(q2cb_ps[:, :], lhsT=ones_row[:1, :], rhs=q2c_sb[:1, :],
                             start=True, stop=True)
            q2cb_sb = bsmall.tile([P, D], f32, tag="q2cb")
            nc.scalar.copy(out=q2cb_sb[:, :], in_=q2cb_ps[:, :])

            hq_all = hqpool.tile([P, 7 * D], f32, tag="hq")
            q2cb_b3 = bass.AP(tensor=q2cb_sb.tensor, offset=q2cb_sb.offset,
                              ap=[q2cb_sb.ap[0], [0, 3], q2cb_sb.ap[1]])
            nc.vector.tensor_mul(
                out=hq_all[:, 0:3 * D].rearrange("p (n d) -> p n d", d=D),
                in0=htile[:, 0:3 * D].rearrange("p (n d) -> p n d", d=D),
                in1=q2cb_b3)
            nc.gpsimd.tensor_mul(
                out=hq_all[:, 3 * D:6 * D].rearrange("p (n d) -> p n d", d=D),
                in0=htile[:, 3 * D:6 * D].rearrange("p (n d) -> p n d", d=D),
                in1=q2cb_b3)
            nc.vector.tensor_mul(out=hq_all[0:T - 6 * P, 6 * D:7 * D],
                                 in0=htile[0:T - 6 * P, 6 * D:7 * D],
                                 in1=q2cb_sb[0:T - 6 * P, :])
            nc.sync.dma_start(
                out=g_out[b, 0:6 * P, 3 * D:4 * D].rearrange("(n p) d -> p n d", p=P),
                in_=hq_all[:, 0:6 * D].rearrange("p (n d) -> p n d", d=D))
            nc.sync.dma_start(out=g_out[b, 6 * P:T, 3 * D:4 * D],
                              in_=hq_all[0:T - 6 * P, 6 * D:7 * D])

        if reps == 1:
            for b in range(BC):
                batch_body(b)
        else:
            with tc.For_i(0, reps, 1):
                for b in range(BC):
                    batch_body(b)

    return nc


def kernel(h, u, w_h, b_h, w_u, b_u, w_hu, b_hu):
    from concourse.bass_utils import run_bass_kernel_spmd

    if "nc" not in _cache:
        nc = _build()
        _split_multi_waits(nc)
        _cache["nc"] = nc
    nc = _cache["nc"]

    h = np.ascontiguousarray(h, dtype=np.float32)
    u = np.ascontiguousarray(u, dtype=np.float32)
    w_h = np.ascontiguousarray(w_h, dtype=np.float32)
    w_u = np.ascontiguousarray(w_u, dtype=np.float32)
    w_hu = np.ascontiguousarray(w_hu, dtype=np.float32)

    core_ids = list(range(NCORES))
    in_maps = []
    for i in core_ids:
        in_maps.append({
            "h": h[i * BC:(i + 1) * BC],
            "u": u[i * BC:(i + 1) * BC],
            "w_h": w_h,
            "w_u": w_u,
            "w_hu": w_hu,
        })
    res = run_bass_kernel_spmd(nc, in_maps, core_ids)
    _cache["last_results"] = res
    return np.concatenate([res.results[i]["g"] for i in core_ids], axis=0)
